# revision 57
# baseline (speedup 1.0000x reference)
"""Trainium2 Bass kernel for a dense transformer block (B=4, T=2048, C=1024, H=16).

Sharding: zero-collective. Each of the 8 cores owns (batch b, fold f):
  core c -> b = c//2, f = c%2.
Queries (1024 per core): fold0 owns token chunks {0, 3}, fold1 owns {1, 2}
(chunks of 512).  Keys are stored PERMUTED per core so the causal structure
looks identical on every core:
  fold0 key order: [c0 c1 c2 c3] (natural),  fold1: [c1 c0 c3 c2].
With q-slot0 = key-positions [0:512) and q-slot1 = positions [1536:2048),
both folds see: slot0 reads key tiles 0..7 (diagonal masks at kt 0..3),
slot1 reads tiles 0..15 (diagonal at kt 12..15).  Fully-invisible tiles
are zeroed via a per-core exp bias column (-1e9); the 4 distinct diagonal
masks are shared constants.  QK^T packs 2 heads per 512-cycle pass via
K=64 row-group concurrency (head A partitions 0:64, head B 64:128); exp
runs once over both heads [128, 2x512].  Softmax denominators ride a
ones-column in V; reciprocals are computed column-wise [128, 32] after a
DRAM-roundtrip transpose, then multiplied back into y row-broadcasts.

Host-side constant folding (weight-only transforms): ln1_g/ln2_g into
W_attn/W_fc, bias vectors b1@W'+b_attn / b2@W'+b_fc precomputed in numpy.
x is layer-normed once on-chip into bf16 xhat; qkv evictions are a single
cast+bias op.  b_proj is pre-added to the residual input; b_fc2 rides a
rank-1 matmul row.
"""
import sys
import math
import contextlib

for _p in ("/opt/trn_rl_repo", "/root/.axon_site/_ro/trn_rl_repo"):
    if _p not in sys.path:
        sys.path.append(_p)

import numpy as np
import ml_dtypes

import concourse.bass as bass
import concourse.bacc as bacc
import concourse.mybir as mybir
import concourse.tile as tile
from concourse.bass_utils import run_bass_kernel_spmd

F32 = mybir.dt.float32
BF16 = mybir.dt.bfloat16
AF = mybir.ActivationFunctionType
OP = mybir.AluOpType
BF = ml_dtypes.bfloat16

B, T, C, H = 4, 2048, 1024, 16
HD = C // H              # 64
EPS = 1e-5
TB = T                   # tokens per batch (2048)
TQ = T // 2              # query tokens per core (1024)
CK = C // 128            # 8 contraction k-tiles over C
NTT = TB // 128          # 16 token tiles per batch
NQT = TQ // 128          # 8 token tiles per core's queries
G = 4                    # head groups (4 heads each)
DG = 256                 # q/k/v cols per group
FC = 4 * C               # 4096
NGT = FC // 128          # 32 fc tiles
NEG = -1e9


def build_nc(dbg=False):
    nc = bacc.Bacc("TRN2", target_bir_lowering=False, debug=False, num_devices=8)

    # all tensors pre-arranged host-side to be contiguous per partition
    dt_in = {
        "xT": ([128, 4, CK, 512], BF16),       # [p, ch, k, t]
        "xn": ([128, 4, 4, C], BF16),          # [p, ch, tl, c]
        "wvw": ([128, 2, CK, 512], BF16),      # [p, slab, k, c]
        "wkqw": ([128, G, CK, 2, DG], BF16),   # [p, g, k, xi, c]
        "wproj": ([128, CK, C], BF16),
        "wfc": ([128, 8, CK, 512], BF16),      # [p, gtg, k, c]
        "wfc2": ([128, 2, 8, 4, 512], BF16),   # [p, n, gkb, gi, c]
        "masks": ([128, 4, 2, 512], BF16),
        "xr": ([128, NQT, C], F32),            # [p, m, c]
        "beta_col": ([128, 16], F32),
        "betav_bc": ([128, C], F32),
        "ebias": ([128, 24], F32),
        "bfc_col": ([128, NGT], F32),
        "bfc2_row": ([1, C], BF16),
    }
    d = {k: nc.dram_tensor(k, sh, dt, kind="ExternalInput").ap()
         for k, (sh, dt) in dt_in.items()}
    out = nc.dram_tensor("out", [TQ, C], F32, kind="ExternalOutput").ap()
    if dbg:
        for k, sh, dt in [("dbg_xhat", [128, CK, TB], BF16),
                          ("dbg_q", [128, 2, TQ], BF16),
                          ("dbg_k", [128, 2, TB], BF16),
                          ("dbg_va", [128, NTT, 8, 65], BF16),
                          ("dbg_y", [128, 8, TQ], BF16),
                          ("dbg_x2", [128, NQT, C], F32)]:
            d[k] = nc.dram_tensor(k, sh, dt, kind="ExternalOutput").ap()

    with tile.TileContext(nc) as tc:
        with contextlib.ExitStack() as ctx:
            _build_body(nc, tc, ctx, d, out, dbg)
    nc.compile()
    return nc


def _build_body(nc, tc, ctx, d, out, dbg=False):
    pool = lambda name, bufs, **kw: ctx.enter_context(
        tc.tile_pool(name=name, bufs=bufs, **kw))

    cons = pool("cons", 1)
    stats = pool("stats", 3)
    ps = pool("ps", 2, space="PSUM")
    dram = pool("dram", 2, space="DRAM")
    yp = pool("yp", 1)
    wpjp = pool("wpjp", 1)

    eps_t = cons.tile([128, 1], F32)
    nc.vector.memset(eps_t, EPS)
    ident = cons.tile([128, 128], BF16)
    from concourse.masks import make_identity
    make_identity(nc, ident)

    beta_sb = cons.tile([128, 16], F32)
    nc.sync.dma_start(out=beta_sb, in_=d["beta_col"])
    betav_sb = cons.tile([128, C], F32)
    nc.sync.dma_start(out=betav_sb, in_=d["betav_bc"])
    ebias_sb = cons.tile([128, 24], F32)
    nc.sync.dma_start(out=ebias_sb, in_=d["ebias"])
    masksb = cons.tile([128, 4, 2, 512], BF16)
    nc.sync.dma_start(out=masksb, in_=d["masks"])

    # ---- phase 0: LN1 stats + xhat normalization (pipelined per 512-chunk) ----
    wpj_t = wpjp.tile([128, CK, C], BF16)

    mid1 = contextlib.ExitStack()   # freed after attention (before proj)
    xh = mid1.enter_context(tc.tile_pool(name="xh", bufs=1))
    # per-chunk tiles so consumers only depend on their own chunk's normalize
    xhat_c = [xh.tile([128, CK, 512], BF16, name=f"xhat{ch}") for ch in range(4)]
    bc = mid1.enter_context(tc.tile_pool(name="bc", bufs=2))

    # ---- attention-scope pools (created early so weight prefetch can start) ----
    attn_ctx = contextlib.ExitStack()
    apool = lambda name, bufs, **kw: attn_ctx.enter_context(
        tc.tile_pool(name=name, bufs=bufs, **kw))
    wvp = apool("wvp", 1)
    wkq = apool("wkq", 2)
    vap = apool("vap", 2)
    qtp = apool("qtp", 2)
    ktp = apool("ktp", 2)
    pp = apool("pp", 6)
    dp = apool("dp", 2)
    rbp = apool("rbp", 4)
    otp = apool("otp", 2)
    ps2 = apool("ps2", 2, space="PSUM")
    psy = apool("psy", 1, space="PSUM")

    def load_wv(slab):
        wv = wvp.tile([128, CK, 512], BF16, name="wv")
        nc.sync.dma_start(out=wv, in_=d["wvw"][:, slab])
        return wv

    def load_wt(g):
        wt = wkq.tile([128, CK, 2, DG], BF16, name="wt")
        nc.sync.dma_start(out=wt, in_=d["wkqw"][:, g])
        return wt

    pre_wv = load_wv(0)
    pre_wt = {0: load_wt(0), 1: load_wt(1)}

    for ch in range(4):
        xhat = xhat_c[ch]
        nc.scalar.dma_start(out=xhat, in_=d["xT"][:, ch])
        xt_f = stats.tile([128, 4, C], BF16, name="xt_f")
        nc.scalar.dma_start(out=xt_f, in_=d["xn"][:, ch])
        mr_cc = stats.tile([128, 8], BF16, name="mr_cc")
        for tl in range(4):
            st = stats.tile([128, 2, 6], F32, name="st")
            resh = xt_f[:, tl, :].rearrange("p (n f) -> p n f", f=512)
            for i in range(2):
                nc.vector.bn_stats(out=st[:, i, :], in_=resh[:, i, :])
            mv = stats.tile([128, 2], F32, name="mv")
            nc.vector.bn_aggr(out=mv, in_=st)
            sd = stats.tile([128, 1], F32, name="sd")
            nc.scalar.activation(sd, mv[:, 1:2], AF.Sqrt, bias=eps_t)
            rf = stats.tile([128, 1], F32, name="rf")
            nc.vector.reciprocal(rf, sd)
            nc.vector.tensor_copy(mr_cc[:, tl:tl + 1], mv[:, 0:1])
            nc.vector.tensor_copy(mr_cc[:, 4 + tl:5 + tl], rf)
        # transpose [128, 8] -> [8, 128] rows, roundtrip to DRAM, bcast-read
        pst = ps.tile([128, 512], F32, name="ps")
        pstv = pst.bitcast(BF16)[0:8, 0:128]
        nc.tensor.transpose(pstv, mr_cc, ident)
        srow = stats.tile([8, 128], BF16, name="srow")
        nc.vector.tensor_copy(srow, pstv)
        mscr = dram.tile([8, 128], BF16, name="mscr")
        nc.sync.dma_start(out=mscr, in_=srow)
        mflat = mscr.rearrange("r q -> (r q)").unsqueeze(0)
        mu_bc = bc.tile([128, 512], BF16, name="mu_bc")
        r_bc = bc.tile([128, 512], BF16, name="r_bc")
        nc.sync.dma_start(out=mu_bc,
                          in_=mflat[:, 0:512].to_broadcast([128, 512]))
        nc.sync.dma_start(out=r_bc,
                          in_=mflat[:, 512:1024].to_broadcast([128, 512]))
        for kt in range(CK):
            nc.vector.tensor_tensor(xhat[:, kt, :], xhat[:, kt, :],
                                    mu_bc, op=OP.subtract)
            nc.vector.tensor_tensor(xhat[:, kt, :], xhat[:, kt, :],
                                    r_bc, op=OP.mult)

    yT2_p = [yp.tile([128, TQ], BF16, name=f"yT2_{p}") for p in range(8)]

    # ---- attention ----

    def v_pass(slab):
        wv = pre_wv if slab == 0 else load_wv(slab)
        va_c = [vap.tile([128, 4, 8, 65], BF16, name=f"va{ch}")
                for ch in range(4)]
        bv = betav_sb[:, slab * 512:(slab + 1) * 512]
        for ch in range(4):
            nc.vector.memset(va_c[ch][:, :, :, 64:65], 1.0)
            for tl in range(4):
                psv = ps.tile([128, 512], F32, name="ps")
                xs = xhat_c[ch]
                tsl = slice(tl * 128, (tl + 1) * 128)
                for kt in range(CK):
                    nc.tensor.matmul(psv, xs[:, kt, tsl], wv[:, kt, :],
                                     start=(kt == 0), stop=(kt == CK - 1))
                nc.vector.tensor_tensor(
                    va_c[ch][:, tl, :, 0:64],
                    psv.rearrange("p (h f) -> p h f", f=64),
                    bv.rearrange("p (h f) -> p h f", f=64), op=OP.add)
        return va_c

    va_slabs = {}
    pending_norm = None
    for g in range(G):
        slab = g // 2
        if g % 2 == 0:
            va_slabs[slab] = v_pass(slab)
        va = va_slabs[slab]

        wt = pre_wt.pop(g) if g in pre_wt else load_wt(g)
        if g == 3:
            nc.sync.dma_start(out=wpj_t, in_=d["wproj"])

        qT_s = [qtp.tile([128, 2, 512], BF16, name=f"qT{slot}")
                for slot in range(2)]
        kT_c = [ktp.tile([128, 2, 512], BF16, name=f"kT{ch}")
                for ch in range(4)]

        def k_ev(j, ch):
            psk = ps.tile([128, 512], F32, name="ps")
            for kt in range(CK):
                nc.tensor.matmul(psk, wt[:, kt, 1, j * 128:(j + 1) * 128],
                                 xhat_c[ch][:, kt, :],
                                 start=(kt == 0), stop=(kt == CK - 1))
            nc.vector.tensor_scalar_add(
                kT_c[ch][:, j, :], in0=psk,
                scalar1=beta_sb[:, 8 + 2 * g + j: 8 + 2 * g + j + 1])

        def q_ev(j, slot):
            psq = ps.tile([128, 512], F32, name="ps")
            xs = xhat_c[0 if slot == 0 else 3]
            for kt in range(CK):
                nc.tensor.matmul(psq, wt[:, kt, 0, j * 128:(j + 1) * 128],
                                 xs[:, kt, :],
                                 start=(kt == 0), stop=(kt == CK - 1))
            nc.vector.tensor_scalar_add(
                qT_s[slot][:, j, :], in0=psq,
                scalar1=beta_sb[:, 2 * g + j: 2 * g + j + 1])

        for ch in (0, 1):
            k_ev(0, ch); k_ev(1, ch)
        q_ev(0, 0); q_ev(1, 0)
        for ch in (2, 3):
            k_ev(0, ch); k_ev(1, ch)
        q_ev(0, 1); q_ev(1, 1)

        if dbg and g == 0:
            for _ch in range(4):
                nc.sync.dma_start(
                    out=d["dbg_xhat"][:, :, _ch * 512:(_ch + 1) * 512],
                    in_=xhat_c[_ch])


        for j in range(2):
            hA = (g % 2) * 4 + 2 * j   # slab-relative head index (even)
            for slot in range(2):
                dscr = dram.tile([2, 512], BF16, name="dscr")
                nkt = 8 if slot == 0 else 16
                pya = psy.tile([65, 512], F32, name="pya")
                pyb = psy.tile([65, 512], F32, name="pyb")
                for kt in range(nkt):
                    p2 = ps2.tile([128, 2, 512], F32, name="p2")
                    kts = kT_c[kt // 4]
                    ksl = slice((kt % 4) * 128, (kt % 4 + 1) * 128)
                    nc.tensor.matmul(p2[:, 0, :],
                                     kts[0:64, j, ksl],
                                     qT_s[slot][0:64, j, :],
                                     start=True, stop=True)
                    nc.tensor.matmul(p2[:, 1, :],
                                     kts[64:128, j, ksl],
                                     qT_s[slot][64:128, j, :],
                                     start=True, stop=True)
                    P2 = pp.tile([128, 2, 512], BF16, name="P2")
                    epos = kt if slot == 0 else 8 + kt
                    nc.scalar.activation(
                        P2.rearrange("p h q -> p (h q)"),
                        p2.rearrange("p h q -> p (h q)"),
                        AF.Exp, scale=1.0 / math.sqrt(HD),
                        bias=ebias_sb[:, epos:epos + 1])
                    mpos = kt if slot == 0 else kt - 12
                    if 0 <= mpos < 4:
                        nc.vector.tensor_mul(P2, P2, masksb[:, mpos, :, :])
                    vas = va[kt // 4]
                    nc.tensor.matmul(pya, vas[:, kt % 4, hA, :], P2[:, 0, :],
                                     start=(kt == 0), stop=(kt == nkt - 1))
                    nc.tensor.matmul(pyb, vas[:, kt % 4, hA + 1, :], P2[:, 1, :],
                                     start=(kt == 0), stop=(kt == nkt - 1))
                # evict unnormalized y (bf16) + denominator rows
                pair = g * 2 + j
                qs = slice(slot * 512, (slot + 1) * 512)
                nc.vector.tensor_copy(yT2_p[pair][0:64, qs], pya[0:64, :])
                otmp = otp.tile([64, 512], BF16, name="otmp")
                nc.vector.tensor_copy(otmp, pyb[0:64, :])
                nc.gpsimd.dma_start(out=yT2_p[pair][64:128, qs], in_=otmp)
                dsb = dp.tile([128, 2, 512], BF16, name="dsb")
                nc.vector.tensor_copy(dsb[64:65, 0, :], pya[64:65, :])
                nc.vector.tensor_copy(dsb[64:65, 1, :], pyb[64:65, :])
                nc.gpsimd.dma_start(out=dscr.rearrange("r q -> (r q)").unsqueeze(0),
                                    in_=dsb[64:65, :, :])

                # columnwise reciprocal of this slot's 2 denominator rows
                # (flat[p*8+i] layout keeps every DMA contiguous per partition)
                dcol = dp.tile([128, 8], BF16, name="dcol")
                nc.gpsimd.dma_start(out=dcol, in_=dscr.rearrange("r q -> (r q)")
                                    .rearrange("(p i) -> p i", p=128))
                rcol = dp.tile([128, 8], BF16, name="rcol")
                with nc.allow_low_precision(reason="softmax denom recip bf16"):
                    nc.vector.reciprocal(rcol, dcol)
                rscr = dram.tile([2, 512], BF16, name="rscr")
                nc.gpsimd.dma_start(out=rscr.rearrange("r q -> (r q)")
                                    .rearrange("(p i) -> p i", p=128), in_=rcol)

                def norm_muls(pair_, slot_, rscr_):
                    qs = slice(slot_ * 512, (slot_ + 1) * 512)
                    for h in range(2):
                        rb_t = rbp.tile([128, 512], BF16, name="rb_t")
                        psl = slice(h * 64, h * 64 + 64)
                        nc.sync.dma_start(
                            out=rb_t[psl, :],
                            in_=rscr_[h:h + 1, :].to_broadcast([64, 512]))
                        nc.vector.tensor_mul(yT2_p[pair_][psl, qs],
                                             yT2_p[pair_][psl, qs], rb_t[psl, :])

                # defer the previous slot's yT2 normalization so the in-order
                # vector queue never stalls on the denominator DMA roundtrip
                if pending_norm is not None:
                    pending_norm()
                pending_norm = (lambda p_=g * 2 + j, s_=slot, r_=rscr:
                                norm_muls(p_, s_, r_))

    pending_norm()
    if dbg:
        for _p in range(8):
            nc.sync.dma_start(out=d["dbg_y"][:, _p, :], in_=yT2_p[_p])
    attn_ctx.close()
    mid1.close()

    # ---- proj + residual ----
    mlp = pool("mlp", 1)
    mstr = pool("mstr", 2)
    wstream = pool("wstream", 3)
    c2 = pool("c2", 1)
    x2_m = [mlp.tile([128, C], F32, name=f"x2_{m}") for m in range(NQT)]

    bfc_sb = c2.tile([128, NGT], F32)
    nc.sync.dma_start(out=bfc_sb, in_=d["bfc_col"])
    ones_row = c2.tile([1, 128], BF16)
    nc.vector.memset(ones_row, 1.0)
    bfc2_sb = c2.tile([1, C], BF16)
    nc.sync.dma_start(out=bfc2_sb, in_=d["bfc2_row"])

    for m in range(NQT):
        xr_t = mstr.tile([128, C], F32, name="xr_t")
        nc.sync.dma_start(out=xr_t, in_=d["xr"][:, m])
        for n in range(2):
            psp = ps.tile([128, 512], F32, name="ps")
            for kt in range(CK):
                nc.tensor.matmul(psp, yT2_p[kt][:, m * 128:(m + 1) * 128],
                                 wpj_t[:, kt, n * 512:(n + 1) * 512],
                                 start=(kt == 0), stop=(kt == CK - 1))
            sl = slice(n * 512, (n + 1) * 512)
            nc.vector.tensor_tensor(x2_m[m][:, sl], psp, xr_t[:, sl], op=OP.add)

    if dbg:
        for _m in range(NQT):
            nc.sync.dma_start(out=d["dbg_x2"][:, _m, :], in_=x2_m[_m])

    # ---- LN2 + transpose (g2/b2 folded into wfc/beta2 host-side) ----
    hT_h = [mlp.tile([128, CK, 512], BF16, name=f"hT{th}") for th in range(2)]
    for m in range(NQT):
        st = stats.tile([128, 2, 6], F32, name="st")
        resh = x2_m[m].rearrange("p (n f) -> p n f", f=512)
        for i in range(2):
            nc.vector.bn_stats(out=st[:, i, :], in_=resh[:, i, :])
        mv = stats.tile([128, 2], F32, name="mv")
        nc.vector.bn_aggr(out=mv, in_=st)
        sd = stats.tile([128, 1], F32, name="sd")
        nc.scalar.activation(sd, mv[:, 1:2], AF.Sqrt, bias=eps_t)
        r2 = stats.tile([128, 1], F32, name="r2")
        nc.vector.reciprocal(r2, sd)
        hmb = mstr.tile([128, C], BF16, name="hmb")
        nc.vector.tensor_scalar(hmb, in0=x2_m[m], scalar1=mv[:, 0:1],
                                scalar2=r2, op0=OP.subtract, op1=OP.mult)
        tsl = slice((m % 4) * 128, (m % 4 + 1) * 128)
        for ck in range(CK):
            pst = ps.tile([128, 512], F32, name="ps")
            pstv = pst.bitcast(BF16)[:, 0:128]
            nc.tensor.transpose(pstv, hmb[:, ck * 128:(ck + 1) * 128], ident)
            nc.scalar.copy(hT_h[m // 4][:, ck, tsl], pstv)

    # ---- MLP ----
    hid = mlp.tile([128, NGT, 512], BF16, name="hid")
    psacc = pool("psacc", 4, space="PSUM")
    ostg = pool("ostg", 3)
    for th in range(2):
        for gtg in range(NGT // 4):
            wfcg = wstream.tile([128, CK, 512], BF16, name="wfcg")
            nc.scalar.dma_start(out=wfcg, in_=d["wfc"][:, gtg])
            for gi in range(4):
                gt = gtg * 4 + gi
                psf = ps.tile([128, 512], F32, name="ps")
                for kt in range(CK):
                    nc.tensor.matmul(psf, wfcg[:, kt, gi * 128:(gi + 1) * 128],
                                     hT_h[th][:, kt, :],
                                     start=(kt == 0), stop=(kt == CK - 1))
                nc.scalar.activation(hid[:, gt, :], psf, AF.Gelu,
                                     bias=bfc_sb[:, gt:gt + 1])
        for n in range(2):
            accs = [psacc.tile([128, 512], F32, name="acc") for _ in range(4)]
            for gkb in range(NGT // 4):
                wf2 = wstream.tile([128, 4, 512], BF16, name="wf2")
                nc.sync.dma_start(out=wf2, in_=d["wfc2"][:, n, gkb])
                for gi in range(4):
                    gkt = gkb * 4 + gi
                    for ml_ in range(4):
                        nc.tensor.matmul(accs[ml_],
                                         hid[:, gkt, ml_ * 128:(ml_ + 1) * 128],
                                         wf2[:, gi, :], start=(gkt == 0), stop=False)
            for ml_ in range(4):
                m = th * 4 + ml_
                nc.tensor.matmul(accs[ml_], ones_row,
                                 bfc2_sb[:, n * 512:(n + 1) * 512],
                                 start=False, stop=True)
                osb = ostg.tile([128, 512], F32, name="osb")
                nc.vector.tensor_tensor(osb, accs[ml_],
                                        x2_m[m][:, n * 512:(n + 1) * 512], op=OP.add)
                nc.sync.dma_start(out=out[m * 128:(m + 1) * 128,
                                          n * 512:(n + 1) * 512],
                                  in_=osb)


def make_masks():
    """[128, 4, 2, 512] bf16 diagonal masks, duplicated across the head dim."""
    m = np.zeros((4, 128, 512), np.float32)
    kk = np.arange(128)[:, None]
    qq = np.arange(512)[None, :]
    for j in range(4):
        m[j] = ((j * 128 + kk) <= qq).astype(np.float32)
    m2 = np.repeat(m[:, :, None, :], 2, axis=2)          # [4, 128, 2, 512]
    return np.ascontiguousarray(m2.transpose(1, 0, 2, 3)).astype(BF)


def make_in_maps(inputs):
    f32 = lambda a: np.asarray(a, dtype=np.float32)
    x = f32(inputs["x"])
    W_attn, b_attn = f32(inputs["W_attn"]), f32(inputs["b_attn"])
    W_proj, b_proj = f32(inputs["W_proj"]), f32(inputs["b_proj"])
    W_fc, b_fc = f32(inputs["W_fc"]), f32(inputs["b_fc"])
    W_fc2, b_fc2 = f32(inputs["W_fc2"]), f32(inputs["b_fc2"])
    g1, b1 = f32(inputs["ln1_g"]), f32(inputs["ln1_b"])
    g2, b2 = f32(inputs["ln2_g"]), f32(inputs["ln2_b"])

    # host-side constant folding (weight-only transforms)
    Wq = W_attn * g1[:, None]                        # [C, 3C]
    beta = b1 @ Wq + b_attn                          # [3C]
    Wf = W_fc * g2[:, None]                          # [C, 4C]
    beta2 = b2 @ Wf + b_fc                           # [4C]

    masks = make_masks()
    ebias = np.zeros((2, 128, 24), np.float32)
    ebias[0, :, 4:8] = NEG          # fold0: slot0 kt4..7 invisible
    ebias[1, :, 16:20] = NEG        # fold1: slot1 kt8..11 invisible

    ctg = np.ascontiguousarray
    wq_r = Wq.reshape(CK, 128, 3 * C).transpose(1, 0, 2)      # [p, k, 3C]
    shared = {
        "wvw": ctg(wq_r[:, :, 2 * C:].reshape(128, CK, 2, 512)
                   .transpose(0, 2, 1, 3)).astype(BF),
        "wkqw": ctg(wq_r[:, :, :2 * C].reshape(128, CK, 2, G, DG)
                    .transpose(0, 3, 1, 2, 4)).astype(BF),
        "wproj": ctg(W_proj.reshape(CK, 128, C).transpose(1, 0, 2)).astype(BF),
        "wfc": ctg(Wf.reshape(CK, 128, FC).transpose(1, 0, 2)
                   .reshape(128, CK, 8, 512).transpose(0, 2, 1, 3)).astype(BF),
        "wfc2": ctg(W_fc2.reshape(8, 4, 128, 2, 512)
                    .transpose(2, 3, 0, 1, 4)).astype(BF),
        "masks": masks,
        "beta_col": ctg(beta[:2 * C].reshape(16, 128).T).astype(np.float32),
        "betav_bc": ctg(np.broadcast_to(beta[2 * C:], (128, C))).astype(np.float32),
        "bfc_col": ctg(beta2.reshape(NGT, 128).T).astype(np.float32),
        "bfc2_row": b_fc2[None, :].astype(BF),
    }
    in_maps = []
    chunk_orders = ([0, 1, 2, 3], [1, 0, 3, 2])
    for c in range(8):
        b, f = c // 2, c % 2
        order = chunk_orders[f]
        xp = np.concatenate([x[b, 512 * ci:512 * (ci + 1)] for ci in order])
        xr = np.concatenate([xp[0:512], xp[1536:2048]]) + b_proj[None, :]
        in_maps.append(dict(
            shared,
            xT=ctg(xp.reshape(4, 512, CK, 128).transpose(3, 0, 2, 1)).astype(BF),
            xn=ctg(xp.reshape(4, 4, 128, C).transpose(2, 0, 1, 3)).astype(BF),
            xr=ctg(xr.reshape(NQT, 128, C).transpose(1, 0, 2)).astype(np.float32),
            ebias=ebias[f],
        ))
    return in_maps


def assemble_out(results):
    out = np.empty((B, T, C), np.float32)
    for c in range(8):
        b, f = c // 2, c % 2
        r = results[c]["out"]
        if f == 0:
            out[b, 0:512] = r[0:512]
            out[b, 1536:2048] = r[512:1024]
        else:
            out[b, 512:1024] = r[0:512]
            out[b, 1024:1536] = r[512:1024]
    return out


_NC_CACHE = {}


def kernel(**inputs):
    if "nc" not in _NC_CACHE:
        _NC_CACHE["nc"] = build_nc()
    nc = _NC_CACHE["nc"]
    in_maps = make_in_maps(inputs)
    rr = run_bass_kernel_spmd(nc, in_maps, list(range(8)))
    return assemble_out(rr.results)
